# revision 51
# baseline (speedup 1.0000x reference)
"""AlignmentEncoder forward on 8 Trainium2 NeuronCores (data-parallel over batch).

Computes, per batch b (one batch per core):
  keys/queries conditioned with speaker+emotion projections,
  keys_enc = conv1d(relu(conv1d(keys, kw1, pad1)), kw2)         (80, 256)
  queries_enc = conv1d(relu(conv1d(relu(conv1d(q,qw1,pad1)),qw2)),qw3)  (80, 1000)
  x[t,s] = -TEMP*||q_t - k_s||^2  (modulo a per-row constant that cancels
           in both log_softmax and softmax: the q2 term is dropped)
  out_logp = x - logsumexp_s(x) + log(prior + 1e-8)
  out_attn = softmax_s(x + log(prior + 1e-8)) with masked s zeroed.

Matmul-heavy work runs in bf16 (weights pre-transposed host-side into lhsT
layouts); softmax epilogue is f32.
"""

import sys
import types
from contextlib import ExitStack

sys.path.insert(0, "/opt/trn_rl_repo")

import numpy as np
import ml_dtypes

import bass_rust
from bass_rust import add_dep_helper
import concourse.bass as bass
import concourse.mybir as mybir
import concourse.tile as tile
from concourse.vector_clock import ScopedClock

BF16 = ml_dtypes.bfloat16
F32 = np.float32

B, MEL, TXT, ATT, T1, T2 = 8, 80, 512, 80, 1000, 256
TEMP = 0.0005
N_CORES = 8

_MAX_WAITS = 1  # this walrus build rejects multi-wait instructions


def _split_excess_waits(nc):
    """Move excess sem waits from any instruction onto same-engine NoOps
    inserted immediately before it (program order on the engine's
    sequencer preserves the wait semantics)."""
    uid = 0
    for blk in nc.m.functions[0].blocks:
        insts = list(blk.instructions)
        out = []
        changed = False
        for inst in insts:
            si = inst.sync_info
            waits = list(si.on_wait) if si is not None and si.on_wait else []
            if len(waits) > _MAX_WAITS:
                si.on_wait = waits[-_MAX_WAITS:]
                extra = waits[: -_MAX_WAITS]
                for j in range(0, len(extra), _MAX_WAITS):
                    nop = mybir.InstNoOp(name=f"I-waitsplit-{uid}", ins=[], outs=[])
                    uid += 1
                    nop.engine = inst.engine
                    nop.bass_nofuse = True
                    nop.sync_info = bass_rust.SyncInfo(
                        on_wait=extra[j : j + _MAX_WAITS], on_update=[]
                    )
                    out.append(nop)
                changed = True
            out.append(inst)
        if changed:
            blk.instructions = out


class _TC(tile.TileContext):
    pass


def _build_nc(salt=""):
    f32 = mybir.dt.float32
    bf16 = mybir.dt.bfloat16
    AF = mybir.ActivationFunctionType
    AX = mybir.AxisListType

    nc = bass.Bass("TRN2", target_bir_lowering=False, debug=False, num_devices=N_CORES)
    dp = nc.declare_dram_parameter
    queries_p = dp("queries", [MEL, T1], bf16, isOutput=False)
    keys_p = dp("keys", [4, 128, T2], bf16, isOutput=False)
    prior_p = dp("prior", [T1, T2], f32, isOutput=False)
    pmask_p = dp("pmask", [T2], f32, isOutput=False)
    z_p = dp("z", [128, 8], bf16, isOutput=False)
    wcat_p = dp("wcat", [128, 8, 592], bf16, isOutput=False)
    bvec_p = dp("bvec", [128, 5], f32, isOutput=False)
    kw1_p = dp("kw1", [128, 8, 4, 3, 128], bf16, isOutput=False)
    kw2_p = dp("kw2", [128, 8, ATT], bf16, isOutput=False)
    kb1_p = dp("kb1", [128, 8], f32, isOutput=False)
    kb2_p = dp("kb2", [ATT, 1], f32, isOutput=False)
    qw1_p = dp("qw1", [MEL, 3, 160], bf16, isOutput=False)
    qw2_p = dp("qw2", [160, MEL], bf16, isOutput=False)
    qw3_p = dp("qw3", [MEL, MEL], bf16, isOutput=False)
    qb1_p = dp("qb1", [160, 1], f32, isOutput=False)
    qb2_p = dp("qb2", [MEL, 1], f32, isOutput=False)
    qb3_p = dp("qb3", [MEL, 1], f32, isOutput=False)
    oattn_p = dp("out_attn", [T1, T2], f32, isOutput=True)
    ologp_p = dp("out_logp", [T1, T2], f32, isOutput=True)

    with _TC(nc) as tc, ExitStack() as ctx:
        const = ctx.enter_context(tc.tile_pool(name="const" + salt, bufs=1))
        ring = ctx.enter_context(tc.tile_pool(name="ring" + salt, bufs=3))
        work = ctx.enter_context(tc.tile_pool(name="work" + salt, bufs=6))
        small = ctx.enter_context(tc.tile_pool(name="small" + salt, bufs=8))

        # ---- constants / inputs to SBUF.  Front-chain inputs first; bulk
        # weights gated behind them so the DMA engines drain the critical
        # chain before starting on conv1's 3.1MB.
        ztile = const.tile([128, 8], bf16, tag="ztile")
        nc.sync.dma_start(out=ztile, in_=z_p[:])
        wcat_sb = const.tile([128, 8, 592], bf16, tag="wcat")
        front_dmas = [
            nc.sync.dma_start(out=wcat_sb[:, 0:4, :], in_=wcat_p[:, 0:4, :]),
            nc.sync.dma_start(out=wcat_sb[:, 4:8, :], in_=wcat_p[:, 4:8, :]),
        ]
        kbuf = const.tile([128, 4, T2 + 2], bf16, tag="kbuf")
        for ci in range(4):
            front_dmas.append(
                nc.sync.dma_start(out=kbuf[:, ci, 1 : T2 + 1], in_=keys_p[ci])
            )
        qbuf = const.tile([MEL, T1 + 2], bf16, tag="qbuf")
        front_dmas.append(nc.sync.dma_start(out=qbuf[:, 1 : T1 + 1], in_=queries_p[:]))
        bvec_sb = const.tile([128, 5], f32, tag="bvec")
        nc.sync.dma_start(out=bvec_sb, in_=bvec_p[:])
        qw1_sb = const.tile([MEL, 3, 160], bf16, tag="qw1")
        nc.sync.dma_start(out=qw1_sb, in_=qw1_p[:])
        qw2a_sb = const.tile([128, MEL], bf16, tag="qw2a")
        nc.sync.dma_start(out=qw2a_sb, in_=qw2_p[0:128, :])
        qw2b_sb = const.tile([32, MEL], bf16, tag="qw2b")
        nc.sync.dma_start(out=qw2b_sb, in_=qw2_p[128:160, :])
        qw3_sb = const.tile([MEL, MEL], bf16, tag="qw3")
        nc.sync.dma_start(out=qw3_sb, in_=qw3_p[:])
        kb1_sb = const.tile([128, 8], f32, tag="kb1")
        nc.sync.dma_start(out=kb1_sb, in_=kb1_p[:])
        kb2_sb = const.tile([ATT, 1], f32, tag="kb2")
        nc.sync.dma_start(out=kb2_sb, in_=kb2_p[:])
        qb1a_sb = const.tile([128, 1], f32, tag="qb1a")
        nc.sync.dma_start(out=qb1a_sb, in_=qb1_p[0:128, :])
        qb1b_sb = const.tile([32, 1], f32, tag="qb1b")
        nc.sync.dma_start(out=qb1b_sb, in_=qb1_p[128:160, :])
        qb2_sb = const.tile([MEL, 1], f32, tag="qb2")
        nc.sync.dma_start(out=qb2_sb, in_=qb2_p[:])
        qb3_sb = const.tile([MEL, 1], f32, tag="qb3")
        nc.sync.dma_start(out=qb3_sb, in_=qb3_p[:])
        # kw1 weight slices: issue only after the latency-critical front inputs
        kw1_slices = []
        kw1_dmas = []
        for co in range(8):
            kw1_sl = ring.tile([128, 4, 3, 128], bf16, tag=f"kw1_{co}", bufs=1)
            d = nc.sync.dma_start(out=kw1_sl, in_=kw1_p[:, co])
            for fd in front_dmas:
                add_dep_helper(d.ins, fd.ins, reason="dma staging: kw1 after front")
            kw1_slices.append(kw1_sl)
            kw1_dmas.append(d)
        # mid-priority loads, needed only for conv2 / the epilogue
        kw2_sb = const.tile([128, 8, ATT], bf16, tag="kw2")
        d = nc.sync.dma_start(out=kw2_sb, in_=kw2_p[:])
        add_dep_helper(d.ins, kw1_dmas[3].ins, reason="dma staging: kw2 after kw1[3]")
        pmask_bc = const.tile([128, T2], f32, tag="pmaskbc")
        pmask_ap = pmask_p[:]
        pmask_bcast = bass.AP(
            tensor=pmask_ap.tensor,
            offset=pmask_ap.offset,
            ap=[[0, 128]] + list(pmask_ap.ap),
        )
        d = nc.sync.dma_start(out=pmask_bc, in_=pmask_bcast)
        add_dep_helper(d.ins, kw1_dmas[3].ins, reason="dma staging: pmask after kw1[3]")
        prior_sb = const.tile([128, 8, T2], f32, tag="priorsb")
        nc.gpsimd.memset(prior_sb, 1e-8)

        ones80 = const.tile([ATT, 1], bf16, tag="ones80")
        nc.vector.memset(ones80, 1.0)
        eps_sb = const.tile([128, 1], f32, tag="eps8")
        nc.vector.memset(eps_sb, 1e-8)
        ones1 = const.tile([1, 128], bf16, tag="ones1")
        nc.vector.memset(ones1, 1.0)
        qb3s_sb = const.tile([MEL, 1], f32, tag="qb3s")
        nc.scalar.mul(out=qb3s_sb, in_=qb3_sb, mul=2.0 * TEMP)

        keys_bf = const.tile([128, 4, T2 + 2], bf16, tag="keysbf")
        queries_bf = const.tile([MEL, T1 + 2], bf16, tag="queriesbf")
        for ci in range(4):
            nc.vector.memset(keys_bf[:, ci, 0:1], 0.0)
            nc.vector.memset(keys_bf[:, ci, T2 + 1 : T2 + 2], 0.0)
        nc.vector.memset(queries_bf[:, 0:1], 0.0)
        nc.vector.memset(queries_bf[:, T1 + 1 : T1 + 2], 0.0)

        keys1_bf = const.tile([128, 8, T2], bf16, tag="keys1")
        Qp = const.tile([MEL, T1], bf16, tag="Qp")
        kenc_sb = const.tile([ATT, T2], bf16, tag="kenc")
        sk_sb = const.tile([ATT, T2], bf16, tag="sk")
        k2neg_sb = const.tile([1, T2], bf16, tag="k2neg")
        q1a_bf = const.tile([128, T1], bf16, tag="q1a")
        q1b_bf = const.tile([32, T1], bf16, tag="q1b")
        q2_bf = const.tile([MEL, T1], bf16, tag="q2bf")

        # ---- stage A: speaker/emotion projections  proj = Wcat @ z + bvec
        warm_w = const.tile([128, 8], bf16, tag="warmw")
        nc.vector.memset(warm_w, 0.0)
        warm_rhs = const.tile([128, 512], bf16, tag="warmrhs")
        nc.vector.memset(warm_rhs, 0.0)
        with tc.tile_pool(name="psA" + salt, bufs=1, space="PSUM") as psA:
            # dummy matmuls to lift the PE HAM clock gate (~3.5us of activity)
            warm_ps = psA.tile([8, 512], f32, tag="warm")
            for _ in range(9):
                nc.tensor.matmul(
                    warm_ps, lhsT=warm_w, rhs=warm_rhs, start=True, stop=True
                )
            proj_ps = psA.tile([128, 5], f32, tag="proj")
            for cb in range(5):
                mw = 128 if cb < 4 else 80
                for jc in range(8):
                    nc.tensor.matmul(
                        proj_ps[0:mw, cb : cb + 1],
                        lhsT=wcat_sb[:, jc, cb * 128 : cb * 128 + mw],
                        rhs=ztile[:, jc : jc + 1],
                        start=(jc == 0),
                        stop=(jc == 7),
                    )
            proj_sb = small.tile([128, 5], f32, tag="proj_sb", bufs=1)
            nc.vector.tensor_add(proj_sb, proj_ps, bvec_sb)
            # keep the PE busy through the conditioning gap so the HAM clock
            # gate stays open for the query/key conv burst
            for _ in range(4):
                nc.tensor.matmul(
                    warm_ps, lhsT=warm_w, rhs=warm_rhs, start=True, stop=True
                )

        # ---- conditioning (adds per-channel projection, casts to bf16)
        nc.vector.tensor_scalar_add(
            queries_bf[:, 1 : T1 + 1], qbuf[:, 1 : T1 + 1], proj_sb[0:MEL, 4:5]
        )
        for ci in range(4):
            nc.vector.tensor_scalar_add(
                keys_bf[:, ci, 1 : T2 + 1],
                kbuf[:, ci, 1 : T2 + 1],
                proj_sb[:, ci : ci + 1],
            )

        # ---- stages B+C interleaved: query conv chain serialises on its
        # relus, so conv1-over-keys co-blocks are emitted into the gaps to
        # keep the PE dense (and its clock un-throttled).
        with (
            tc.tile_pool(name="psB" + salt, bufs=1, space="PSUM") as psB,
            tc.tile_pool(name="psC" + salt, bufs=2, space="PSUM") as psC,
        ):
            def q1_block(t0):
                q1a_ps = psB.tile([128, 500], f32, tag="q1a")
                q1b_ps = psB.tile([32, 500], f32, tag="q1b")
                for k in range(3):
                    nc.tensor.matmul(
                        q1a_ps,
                        lhsT=qw1_sb[:, k, 0:128],
                        rhs=queries_bf[:, t0 + k : t0 + k + 500],
                        start=(k == 0),
                        stop=(k == 2),
                    )
                for k in range(3):
                    nc.tensor.matmul(
                        q1b_ps,
                        lhsT=qw1_sb[:, k, 128:160],
                        rhs=queries_bf[:, t0 + k : t0 + k + 500],
                        start=(k == 0),
                        stop=(k == 2),
                    )
                nc.vector.tensor_scalar(
                    q1a_bf[:, t0 : t0 + 500], q1a_ps, qb1a_sb, 0.0,
                    op0=mybir.AluOpType.add, op1=mybir.AluOpType.max,
                )
                nc.scalar.activation(
                    q1b_bf[:, t0 : t0 + 500], q1b_ps, AF.Relu, bias=qb1b_sb
                )

            def q2_block(t0):
                q2_ps = psB.tile([MEL, 500], f32, tag="q2")
                nc.tensor.matmul(
                    q2_ps, lhsT=qw2a_sb, rhs=q1a_bf[:, t0 : t0 + 500],
                    start=True, stop=False,
                )
                nc.tensor.matmul(
                    q2_ps, lhsT=qw2b_sb, rhs=q1b_bf[:, t0 : t0 + 500],
                    start=False, stop=True,
                )
                nc.scalar.activation(
                    q2_bf[:, t0 : t0 + 500], q2_ps, AF.Relu, bias=qb2_sb
                )

            def q3_block(t0):
                q3_ps = psB.tile([MEL, 500], f32, tag="q3")
                nc.tensor.matmul(
                    q3_ps, lhsT=qw3_sb, rhs=q2_bf[:, t0 : t0 + 500],
                    start=True, stop=True,
                )
                nc.scalar.activation(
                    Qp[:, t0 : t0 + 500], q3_ps, AF.Identity,
                    bias=qb3s_sb, scale=2.0 * TEMP,
                )

            k_ps = psC.tile([ATT, T2], f32, tag="kps", bufs=1)

            def conv1_block(co):
                kw1_sl = kw1_slices[co]
                c1_ps = psC.tile([128, T2], f32, tag="c1")
                n = 0
                for ci in range(4):
                    for k in range(3):
                        nc.tensor.matmul(
                            c1_ps,
                            lhsT=kw1_sl[:, ci, k, :],
                            rhs=keys_bf[:, ci, k : k + T2],
                            start=(n == 0),
                            stop=(n == 11),
                        )
                        n += 1
                if co % 2 == 0:
                    nc.scalar.activation(
                        keys1_bf[:, co, :], c1_ps, AF.Relu,
                        bias=kb1_sb[:, co : co + 1],
                    )
                else:
                    nc.vector.tensor_scalar(
                        keys1_bf[:, co, :],
                        c1_ps,
                        kb1_sb[:, co : co + 1],
                        0.0,
                        op0=mybir.AluOpType.add,
                        op1=mybir.AluOpType.max,
                    )
                nc.tensor.matmul(
                    k_ps,
                    lhsT=kw2_sb[:, co, :],
                    rhs=keys1_bf[:, co, :],
                    start=(co == 0),
                    stop=(co == 7),
                )

            q1_block(0)
            conv1_block(0)
            q1_block(500)
            conv1_block(1)
            q2_block(0)
            conv1_block(2)
            q2_block(500)
            conv1_block(3)
            q3_block(0)
            conv1_block(4)
            q3_block(500)
            conv1_block(5)
            conv1_block(6)
            conv1_block(7)

            nc.scalar.activation(kenc_sb, k_ps, AF.Identity, bias=kb2_sb)
            nc.scalar.activation(sk_sb, k_ps, AF.Square, bias=kb2_sb)

        # prior + 1e-8 via DMA CCE accumulate, gated behind all kw1 loads
        for c in range(8):
            rows = 128 if c < 7 else T1 - 7 * 128
            d = nc.gpsimd.dma_start(
                out=prior_sb[0:rows, c, :],
                in_=prior_p[c * 128 : c * 128 + rows, :],
                accum_op=mybir.AluOpType.add,
            )
            add_dep_helper(
                d.ins, kw1_dmas[7].ins, reason="dma staging: prior after kw1"
            )

        # ---- stage D: attention scores + double softmax epilogue.
        # All 8 qk matmuls issue as soon as kenc lands (their accumulation
        # groups stay open); the k2 rank-1 correction closes each group once
        # the Square/k2 chain has caught up.
        with tc.tile_pool(name="psD" + salt, bufs=8, space="PSUM") as psD:
            a_tiles = []
            for c in range(8):
                rows = 128 if c < 7 else T1 - 7 * 128
                a_ps = psD.tile([128, T2], f32, tag="attn", bufs=7)
                nc.tensor.matmul(
                    a_ps[0:rows],
                    lhsT=Qp[:, c * 128 : c * 128 + rows],
                    rhs=kenc_sb,
                    start=True,
                    stop=False,
                )
                a_tiles.append(a_ps)
            with tc.tile_pool(name="psK" + salt, bufs=1, space="PSUM") as psK:
                k2_ps = psK.tile([1, T2], f32, tag="k2")
                nc.tensor.matmul(k2_ps, lhsT=ones80, rhs=sk_sb, start=True, stop=True)
                nc.scalar.activation(k2neg_sb, k2_ps, AF.Copy, scale=-TEMP)
            for c in range(8):
                rows = 128 if c < 7 else T1 - 7 * 128
                r0 = c * 128
                a_ps = a_tiles[c]
                nc.tensor.matmul(
                    a_ps[0:rows],
                    lhsT=ones1[:, 0:rows],
                    rhs=k2neg_sb,
                    start=False,
                    stop=True,
                )
                e = work.tile([128, T2], f32, tag="e")
                s = small.tile([128, 1], f32, tag="s")
                nc.scalar.activation(
                    e[0:rows], a_ps[0:rows], AF.Exp, accum_out=s[0:rows]
                )
                r = small.tile([128, 1], f32, tag="r")
                nc.vector.reciprocal(r[0:rows], s[0:rows])
                ep = work.tile([128, T2], f32, tag="ep")
                nc.vector.tensor_mul(ep[0:rows], e[0:rows], prior_sb[0:rows, c, :])
                o1 = work.tile([128, T2], f32, tag="o1")
                nc.scalar.activation(o1[0:rows], ep[0:rows], AF.Ln, scale=r[0:rows])
                nc.sync.dma_start(out=ologp_p[r0 : r0 + rows, :], in_=o1[0:rows])
                wm = work.tile([128, T2], f32, tag="wm")
                nc.gpsimd.tensor_mul(wm[0:rows], ep[0:rows], pmask_bc[0:rows])
                s2 = small.tile([128, 1], f32, tag="s2")
                nc.vector.reduce_sum(s2[0:rows], wm[0:rows], axis=mybir.AxisListType.X)
                r2 = small.tile([128, 1], f32, tag="r2")
                nc.vector.reciprocal(r2[0:rows], s2[0:rows])
                o2 = work.tile([128, T2], f32, tag="o2")
                r2b = bass.AP(
                    tensor=r2.tensor,
                    offset=r2.offset,
                    ap=[list(r2.ap[0])[:1] + [rows], [0, T2]],
                )
                nc.vector.tensor_mul(o2[0:rows], wm[0:rows], r2b)
                nc.sync.dma_start(out=oattn_p[r0 : r0 + rows, :], in_=o2[0:rows])

    _split_excess_waits(nc)
    return nc


_NC_CACHE = {}


def _get_nc():
    if "nc" not in _NC_CACHE:
        _NC_CACHE["nc"] = _build_nc()
    return _NC_CACHE["nc"]


def _prep_in_maps(inputs):
    q = np.asarray(inputs["queries"], F32)
    k = np.asarray(inputs["keys"], F32)
    mask = np.asarray(inputs["mask"])
    prior = np.asarray(inputs["attn_prior"], F32)
    spk = np.asarray(inputs["speaker_embed"], F32)
    emo = np.asarray(inputs["emotion_embed"], F32)

    kw1 = np.asarray(inputs["kw1"], F32)
    kb1 = np.asarray(inputs["kb1"], F32)
    kw2 = np.asarray(inputs["kw2"], F32)
    kb2 = np.asarray(inputs["kb2"], F32)
    qw1 = np.asarray(inputs["qw1"], F32)
    qb1 = np.asarray(inputs["qb1"], F32)
    qw2 = np.asarray(inputs["qw2"], F32)
    qb2 = np.asarray(inputs["qb2"], F32)
    qw3 = np.asarray(inputs["qw3"], F32)
    qb3 = np.asarray(inputs["qb3"], F32)
    spk_kw = np.asarray(inputs["spk_kw"], F32)
    spk_kb = np.asarray(inputs["spk_kb"], F32)
    spk_qw = np.asarray(inputs["spk_qw"], F32)
    spk_qb = np.asarray(inputs["spk_qb"], F32)
    emo_kw = np.asarray(inputs["emo_kw"], F32)
    emo_kb = np.asarray(inputs["emo_kb"], F32)
    emo_qw = np.asarray(inputs["emo_qw"], F32)
    emo_qb = np.asarray(inputs["emo_qb"], F32)

    # concatenated speaker/emotion projection:  [k_add; q_add] = Wcat @ [spk; emo] + bvec
    wcat = np.concatenate(
        [
            np.concatenate([spk_kw, emo_kw], axis=1),
            np.concatenate([spk_qw, emo_qw], axis=1),
        ],
        axis=0,
    )  # (592, 1024)
    wcat_t = np.ascontiguousarray(
        wcat.T.reshape(8, 128, 592).transpose(1, 0, 2)
    ).astype(BF16)
    bcat = np.zeros(640, F32)
    bcat[0:TXT] = spk_kb + emo_kb
    bcat[TXT : TXT + MEL] = spk_qb + emo_qb
    bvec_t = np.ascontiguousarray(bcat.reshape(5, 128).T)
    z_all = np.concatenate([spk, emo], axis=1)  # (8, 1024)

    kw1_t = np.ascontiguousarray(
        kw1.reshape(8, 128, 4, 128, 3).transpose(3, 0, 2, 4, 1)
    ).astype(BF16)  # (j, co, ci, k, c)
    kw2_t = np.ascontiguousarray(
        kw2[:, :, 0].T.reshape(8, 128, ATT).transpose(1, 0, 2)
    ).astype(BF16)  # (j, ci2, c)
    kb1_t = np.ascontiguousarray(kb1.reshape(8, 128).T)
    qw1_t = np.ascontiguousarray(qw1.transpose(1, 2, 0)).astype(BF16)  # (j, k, c)
    qw2_t = np.ascontiguousarray(qw2[:, :, 0].T).astype(BF16)  # (160, 80)
    qw3_t = np.ascontiguousarray(qw3[:, :, 0].T).astype(BF16)  # (80, 80)

    shared = {
        "wcat": wcat_t,
        "bvec": bvec_t,
        "kw1": kw1_t,
        "kw2": kw2_t,
        "kb1": kb1_t,
        "kb2": np.ascontiguousarray(kb2.reshape(ATT, 1)),
        "qw1": qw1_t,
        "qw2": qw2_t,
        "qw3": qw3_t,
        "qb1": np.ascontiguousarray(qb1.reshape(160, 1)),
        "qb2": np.ascontiguousarray(qb2.reshape(MEL, 1)),
        "qb3": np.ascontiguousarray(qb3.reshape(MEL, 1)),
    }

    in_maps = []
    for b in range(B):
        m = dict(shared)
        m["queries"] = np.ascontiguousarray(q[b]).astype(BF16)
        m["keys"] = np.ascontiguousarray(k[b].reshape(4, 128, T2)).astype(BF16)
        m["prior"] = np.ascontiguousarray(prior[b])
        m["pmask"] = np.ascontiguousarray(
            1.0 - mask[b, :, 0].astype(F32)
        )
        m["z"] = np.ascontiguousarray(z_all[b].reshape(8, 128).T).astype(BF16)
        in_maps.append(m)
    return in_maps


def kernel(**inputs):
    from concourse.bass_utils import run_bass_kernel_spmd

    nc = _get_nc()
    in_maps = _prep_in_maps(inputs)
    res = run_bass_kernel_spmd(nc, in_maps, core_ids=list(range(N_CORES)))
    attn = np.stack([res.results[i]["out_attn"] for i in range(N_CORES)])
    logp = np.stack([res.results[i]["out_logp"] for i in range(N_CORES)])
    return attn[:, None].astype(F32), logp[:, None].astype(F32)


# revision 52
# speedup vs baseline: 1.0445x; 1.0445x over previous
"""AlignmentEncoder forward on 8 Trainium2 NeuronCores (data-parallel over batch).

Computes, per batch b (one batch per core):
  keys/queries conditioned with speaker+emotion projections,
  keys_enc = conv1d(relu(conv1d(keys, kw1, pad1)), kw2)         (80, 256)
  queries_enc = conv1d(relu(conv1d(relu(conv1d(q,qw1,pad1)),qw2)),qw3)  (80, 1000)
  x[t,s] = -TEMP*||q_t - k_s||^2  (modulo a per-row constant that cancels
           in both log_softmax and softmax: the q2 term is dropped)
  out_logp = x - logsumexp_s(x) + log(prior + 1e-8)
  out_attn = softmax_s(x + log(prior + 1e-8)) with masked s zeroed.

Matmul-heavy work runs in bf16 (weights pre-transposed host-side into lhsT
layouts); softmax epilogue is f32.
"""

import sys
import types
from contextlib import ExitStack

sys.path.insert(0, "/opt/trn_rl_repo")

import numpy as np
import ml_dtypes

import bass_rust
from bass_rust import add_dep_helper
import concourse.bass as bass
import concourse.mybir as mybir
import concourse.tile as tile
from concourse.vector_clock import ScopedClock

BF16 = ml_dtypes.bfloat16
F32 = np.float32

B, MEL, TXT, ATT, T1, T2 = 8, 80, 512, 80, 1000, 256
TEMP = 0.0005
N_CORES = 8

_MAX_WAITS = 1  # this walrus build rejects multi-wait instructions


def _split_excess_waits(nc):
    """Move excess sem waits from any instruction onto same-engine NoOps
    inserted immediately before it (program order on the engine's
    sequencer preserves the wait semantics)."""
    uid = 0
    for blk in nc.m.functions[0].blocks:
        insts = list(blk.instructions)
        out = []
        changed = False
        for inst in insts:
            si = inst.sync_info
            waits = list(si.on_wait) if si is not None and si.on_wait else []
            if len(waits) > _MAX_WAITS:
                si.on_wait = waits[-_MAX_WAITS:]
                extra = waits[: -_MAX_WAITS]
                for j in range(0, len(extra), _MAX_WAITS):
                    nop = mybir.InstNoOp(name=f"I-waitsplit-{uid}", ins=[], outs=[])
                    uid += 1
                    nop.engine = inst.engine
                    nop.bass_nofuse = True
                    nop.sync_info = bass_rust.SyncInfo(
                        on_wait=extra[j : j + _MAX_WAITS], on_update=[]
                    )
                    out.append(nop)
                changed = True
            out.append(inst)
        if changed:
            blk.instructions = out


class _TC(tile.TileContext):
    pass


def _build_nc(salt=""):
    f32 = mybir.dt.float32
    bf16 = mybir.dt.bfloat16
    AF = mybir.ActivationFunctionType
    AX = mybir.AxisListType

    nc = bass.Bass("TRN2", target_bir_lowering=False, debug=False, num_devices=N_CORES)
    dp = nc.declare_dram_parameter
    queries_p = dp("queries", [MEL, T1], bf16, isOutput=False)
    keys_p = dp("keys", [4, 128, T2], bf16, isOutput=False)
    prior_p = dp("prior", [T1, T2], f32, isOutput=False)
    pmask_p = dp("pmask", [T2], f32, isOutput=False)
    z_p = dp("z", [128, 8], bf16, isOutput=False)
    wcat_p = dp("wcat", [128, 8, 592], bf16, isOutput=False)
    bvec_p = dp("bvec", [128, 5], f32, isOutput=False)
    kw1_p = dp("kw1", [128, 8, 4, 3, 128], bf16, isOutput=False)
    kw2_p = dp("kw2", [128, 8, ATT], bf16, isOutput=False)
    kb1_p = dp("kb1", [128, 8], f32, isOutput=False)
    kb2_p = dp("kb2", [ATT, 1], f32, isOutput=False)
    qw1_p = dp("qw1", [MEL, 3, 160], bf16, isOutput=False)
    qw2_p = dp("qw2", [160, MEL], bf16, isOutput=False)
    qw3_p = dp("qw3", [MEL, MEL], bf16, isOutput=False)
    qb1_p = dp("qb1", [160, 1], f32, isOutput=False)
    qb2_p = dp("qb2", [MEL, 1], f32, isOutput=False)
    qb3_p = dp("qb3", [MEL, 1], f32, isOutput=False)
    oattn_p = dp("out_attn", [T1, T2], f32, isOutput=True)
    ologp_p = dp("out_logp", [T1, T2], f32, isOutput=True)

    with _TC(nc) as tc, ExitStack() as ctx:
        const = ctx.enter_context(tc.tile_pool(name="const" + salt, bufs=1))
        ring = ctx.enter_context(tc.tile_pool(name="ring" + salt, bufs=3))
        work = ctx.enter_context(tc.tile_pool(name="work" + salt, bufs=6))
        small = ctx.enter_context(tc.tile_pool(name="small" + salt, bufs=8))

        # ---- constants / inputs to SBUF.  Front-chain inputs first; bulk
        # weights gated behind them so the DMA engines drain the critical
        # chain before starting on conv1's 3.1MB.
        ztile = const.tile([128, 8], bf16, tag="ztile")
        nc.sync.dma_start(out=ztile, in_=z_p[:])
        wcat_sb = const.tile([128, 8, 592], bf16, tag="wcat")
        front_dmas = [
            nc.sync.dma_start(out=wcat_sb[:, 0:4, :], in_=wcat_p[:, 0:4, :]),
            nc.sync.dma_start(out=wcat_sb[:, 4:8, :], in_=wcat_p[:, 4:8, :]),
        ]
        kbuf = const.tile([128, 4, T2 + 2], bf16, tag="kbuf")
        for ci in range(4):
            front_dmas.append(
                nc.sync.dma_start(out=kbuf[:, ci, 1 : T2 + 1], in_=keys_p[ci])
            )
        qbuf = const.tile([MEL, T1 + 2], bf16, tag="qbuf")
        front_dmas.append(nc.sync.dma_start(out=qbuf[:, 1 : T1 + 1], in_=queries_p[:]))
        bvec_sb = const.tile([128, 5], f32, tag="bvec")
        nc.sync.dma_start(out=bvec_sb, in_=bvec_p[:])
        qw1_sb = const.tile([MEL, 3, 160], bf16, tag="qw1")
        nc.sync.dma_start(out=qw1_sb, in_=qw1_p[:])
        qw2a_sb = const.tile([128, MEL], bf16, tag="qw2a")
        nc.sync.dma_start(out=qw2a_sb, in_=qw2_p[0:128, :])
        qw2b_sb = const.tile([32, MEL], bf16, tag="qw2b")
        nc.sync.dma_start(out=qw2b_sb, in_=qw2_p[128:160, :])
        qw3_sb = const.tile([MEL, MEL], bf16, tag="qw3")
        nc.sync.dma_start(out=qw3_sb, in_=qw3_p[:])
        kb1_sb = const.tile([128, 8], f32, tag="kb1")
        nc.sync.dma_start(out=kb1_sb, in_=kb1_p[:])
        kb2_sb = const.tile([ATT, 1], f32, tag="kb2")
        nc.sync.dma_start(out=kb2_sb, in_=kb2_p[:])
        qb1a_sb = const.tile([128, 1], f32, tag="qb1a")
        nc.sync.dma_start(out=qb1a_sb, in_=qb1_p[0:128, :])
        qb1b_sb = const.tile([32, 1], f32, tag="qb1b")
        nc.sync.dma_start(out=qb1b_sb, in_=qb1_p[128:160, :])
        qb2_sb = const.tile([MEL, 1], f32, tag="qb2")
        nc.sync.dma_start(out=qb2_sb, in_=qb2_p[:])
        qb3_sb = const.tile([MEL, 1], f32, tag="qb3")
        nc.sync.dma_start(out=qb3_sb, in_=qb3_p[:])
        # kw1 weight slices: issue only after the latency-critical front inputs
        kw1_slices = []
        kw1_dmas = []
        for co in range(8):
            kw1_sl = ring.tile([128, 4, 3, 128], bf16, tag=f"kw1_{co}", bufs=1)
            d = nc.sync.dma_start(out=kw1_sl, in_=kw1_p[:, co])
            for fd in front_dmas:
                add_dep_helper(d.ins, fd.ins, reason="dma staging: kw1 after front")
            kw1_slices.append(kw1_sl)
            kw1_dmas.append(d)
        # mid-priority loads, needed only for conv2 / the epilogue
        kw2_sb = const.tile([128, 8, ATT], bf16, tag="kw2")
        d = nc.sync.dma_start(out=kw2_sb, in_=kw2_p[:])
        add_dep_helper(d.ins, kw1_dmas[3].ins, reason="dma staging: kw2 after kw1[3]")
        pmask_bc = const.tile([128, T2], f32, tag="pmaskbc")
        pmask_ap = pmask_p[:]
        pmask_bcast = bass.AP(
            tensor=pmask_ap.tensor,
            offset=pmask_ap.offset,
            ap=[[0, 128]] + list(pmask_ap.ap),
        )
        d = nc.sync.dma_start(out=pmask_bc, in_=pmask_bcast)
        add_dep_helper(d.ins, kw1_dmas[3].ins, reason="dma staging: pmask after kw1[3]")
        prior_sb = const.tile([128, 8, T2], f32, tag="priorsb")
        nc.gpsimd.memset(prior_sb, 1e-8)

        ones80 = const.tile([ATT, 1], bf16, tag="ones80")
        nc.vector.memset(ones80, 1.0)
        eps_sb = const.tile([128, 1], f32, tag="eps8")
        nc.vector.memset(eps_sb, 1e-8)
        ones1 = const.tile([1, 128], bf16, tag="ones1")
        nc.vector.memset(ones1, 1.0)
        qb3s_sb = const.tile([MEL, 1], f32, tag="qb3s")
        nc.scalar.mul(out=qb3s_sb, in_=qb3_sb, mul=2.0 * TEMP)

        keys_bf = const.tile([128, 4, T2 + 2], bf16, tag="keysbf")
        queries_bf = const.tile([MEL, T1 + 2], bf16, tag="queriesbf")
        for ci in range(4):
            nc.vector.memset(keys_bf[:, ci, 0:1], 0.0)
            nc.vector.memset(keys_bf[:, ci, T2 + 1 : T2 + 2], 0.0)
        nc.vector.memset(queries_bf[:, 0:1], 0.0)
        nc.vector.memset(queries_bf[:, T1 + 1 : T1 + 2], 0.0)

        keys1_bf = const.tile([128, 8, T2], bf16, tag="keys1")
        Qp = const.tile([MEL, T1], bf16, tag="Qp")
        kenc_sb = const.tile([ATT, T2], bf16, tag="kenc")
        sk_sb = const.tile([ATT, T2], bf16, tag="sk")
        k2neg_sb = const.tile([1, T2], bf16, tag="k2neg")
        q1a_bf = const.tile([128, T1], bf16, tag="q1a")
        q1b_bf = const.tile([32, T1], bf16, tag="q1b")
        q2_bf = const.tile([MEL, T1], bf16, tag="q2bf")

        # ---- stage A: speaker/emotion projections  proj = Wcat @ z + bvec
        warm_w = const.tile([128, 8], bf16, tag="warmw")
        nc.vector.memset(warm_w, 0.0)
        warm_rhs = const.tile([128, 512], bf16, tag="warmrhs")
        nc.vector.memset(warm_rhs, 0.0)
        with tc.tile_pool(name="psA" + salt, bufs=1, space="PSUM") as psA:
            # dummy matmuls to lift the PE HAM clock gate (~3.5us of activity)
            warm_ps = psA.tile([8, 512], f32, tag="warm")
            for _ in range(9):
                nc.tensor.matmul(
                    warm_ps, lhsT=warm_w, rhs=warm_rhs, start=True, stop=True
                )
            proj_ps = psA.tile([128, 5], f32, tag="proj")
            for cb in range(5):
                mw = 128 if cb < 4 else 80
                for jc in range(8):
                    nc.tensor.matmul(
                        proj_ps[0:mw, cb : cb + 1],
                        lhsT=wcat_sb[:, jc, cb * 128 : cb * 128 + mw],
                        rhs=ztile[:, jc : jc + 1],
                        start=(jc == 0),
                        stop=(jc == 7),
                    )
            proj_sb = small.tile([128, 5], f32, tag="proj_sb", bufs=1)
            nc.vector.tensor_add(proj_sb, proj_ps, bvec_sb)
            # keep the PE busy through the conditioning gap so the HAM clock
            # gate stays open for the query/key conv burst
            for _ in range(4):
                nc.tensor.matmul(
                    warm_ps, lhsT=warm_w, rhs=warm_rhs, start=True, stop=True
                )

        # ---- conditioning (adds per-channel projection, casts to bf16)
        nc.vector.tensor_scalar_add(
            queries_bf[:, 1 : T1 + 1], qbuf[:, 1 : T1 + 1], proj_sb[0:MEL, 4:5]
        )
        for ci in range(4):
            nc.vector.tensor_scalar_add(
                keys_bf[:, ci, 1 : T2 + 1],
                kbuf[:, ci, 1 : T2 + 1],
                proj_sb[:, ci : ci + 1],
            )

        # ---- stages B+C interleaved: query conv chain serialises on its
        # relus, so conv1-over-keys co-blocks are emitted into the gaps to
        # keep the PE dense (and its clock un-throttled).
        with (
            tc.tile_pool(name="psB" + salt, bufs=1, space="PSUM") as psB,
            tc.tile_pool(name="psC" + salt, bufs=2, space="PSUM") as psC,
        ):
            def q1_block(t0):
                q1a_ps = psB.tile([128, 500], f32, tag="q1a")
                q1b_ps = psB.tile([32, 500], f32, tag="q1b")
                for k in range(3):
                    nc.tensor.matmul(
                        q1a_ps,
                        lhsT=qw1_sb[:, k, 0:128],
                        rhs=queries_bf[:, t0 + k : t0 + k + 500],
                        start=(k == 0),
                        stop=(k == 2),
                    )
                for k in range(3):
                    nc.tensor.matmul(
                        q1b_ps,
                        lhsT=qw1_sb[:, k, 128:160],
                        rhs=queries_bf[:, t0 + k : t0 + k + 500],
                        start=(k == 0),
                        stop=(k == 2),
                    )
                nc.vector.tensor_scalar(
                    q1a_bf[:, t0 : t0 + 500], q1a_ps, qb1a_sb, 0.0,
                    op0=mybir.AluOpType.add, op1=mybir.AluOpType.max,
                )
                nc.scalar.activation(
                    q1b_bf[:, t0 : t0 + 500], q1b_ps, AF.Relu, bias=qb1b_sb
                )

            def q2_block(t0):
                q2_ps = psB.tile([MEL, 500], f32, tag="q2")
                nc.tensor.matmul(
                    q2_ps, lhsT=qw2a_sb, rhs=q1a_bf[:, t0 : t0 + 500],
                    start=True, stop=False,
                )
                nc.tensor.matmul(
                    q2_ps, lhsT=qw2b_sb, rhs=q1b_bf[:, t0 : t0 + 500],
                    start=False, stop=True,
                )
                nc.scalar.activation(
                    q2_bf[:, t0 : t0 + 500], q2_ps, AF.Relu, bias=qb2_sb
                )

            def q3_block(t0):
                q3_ps = psB.tile([MEL, 500], f32, tag="q3")
                nc.tensor.matmul(
                    q3_ps, lhsT=qw3_sb, rhs=q2_bf[:, t0 : t0 + 500],
                    start=True, stop=True,
                )
                nc.scalar.activation(
                    Qp[:, t0 : t0 + 500], q3_ps, AF.Identity,
                    bias=qb3s_sb, scale=2.0 * TEMP,
                )

            k_ps = psC.tile([ATT, T2], f32, tag="kps", bufs=1)

            def conv1_block(co):
                kw1_sl = kw1_slices[co]
                c1_ps = psC.tile([128, T2], f32, tag="c1")
                n = 0
                for ci in range(4):
                    for k in range(3):
                        nc.tensor.matmul(
                            c1_ps,
                            lhsT=kw1_sl[:, ci, k, :],
                            rhs=keys_bf[:, ci, k : k + T2],
                            start=(n == 0),
                            stop=(n == 11),
                        )
                        n += 1
                if co % 2 == 0:
                    nc.scalar.activation(
                        keys1_bf[:, co, :], c1_ps, AF.Relu,
                        bias=kb1_sb[:, co : co + 1],
                    )
                else:
                    nc.vector.tensor_scalar(
                        keys1_bf[:, co, :],
                        c1_ps,
                        kb1_sb[:, co : co + 1],
                        0.0,
                        op0=mybir.AluOpType.add,
                        op1=mybir.AluOpType.max,
                    )
                nc.tensor.matmul(
                    k_ps,
                    lhsT=kw2_sb[:, co, :],
                    rhs=keys1_bf[:, co, :],
                    start=(co == 0),
                    stop=(co == 7),
                )

            q1_block(0)
            conv1_block(0)
            q1_block(500)
            conv1_block(1)
            q2_block(0)
            conv1_block(2)
            q2_block(500)
            conv1_block(3)
            q3_block(0)
            conv1_block(4)
            q3_block(500)
            conv1_block(5)
            conv1_block(6)
            conv1_block(7)

            nc.vector.tensor_scalar_add(kenc_sb, k_ps, kb2_sb)
            nc.scalar.activation(sk_sb, k_ps, AF.Square, bias=kb2_sb)

        # prior + 1e-8 via DMA CCE accumulate, gated behind all kw1 loads
        for c in range(8):
            rows = 128 if c < 7 else T1 - 7 * 128
            d = nc.gpsimd.dma_start(
                out=prior_sb[0:rows, c, :],
                in_=prior_p[c * 128 : c * 128 + rows, :],
                accum_op=mybir.AluOpType.add,
            )
            add_dep_helper(
                d.ins, kw1_dmas[7].ins, reason="dma staging: prior after kw1"
            )

        # ---- stage D: attention scores + double softmax epilogue.
        # All 8 qk matmuls issue as soon as kenc lands (their accumulation
        # groups stay open); the k2 rank-1 correction closes each group once
        # the Square/k2 chain has caught up.
        with tc.tile_pool(name="psD" + salt, bufs=8, space="PSUM") as psD:
            chunk_order = [7, 0, 1, 2, 3, 4, 5, 6]
            a_tiles = {}
            for c in chunk_order:
                rows = 128 if c < 7 else T1 - 7 * 128
                a_ps = psD.tile([128, T2], f32, tag="attn", bufs=7)
                nc.tensor.matmul(
                    a_ps[0:rows],
                    lhsT=Qp[:, c * 128 : c * 128 + rows],
                    rhs=kenc_sb,
                    start=True,
                    stop=False,
                )
                a_tiles[c] = a_ps
            with tc.tile_pool(name="psK" + salt, bufs=1, space="PSUM") as psK:
                k2_ps = psK.tile([1, T2], f32, tag="k2")
                nc.tensor.matmul(k2_ps, lhsT=ones80, rhs=sk_sb, start=True, stop=True)
                nc.scalar.activation(k2neg_sb, k2_ps, AF.Copy, scale=-TEMP)
            for c in chunk_order:
                rows = 128 if c < 7 else T1 - 7 * 128
                r0 = c * 128
                a_ps = a_tiles[c]
                nc.tensor.matmul(
                    a_ps[0:rows],
                    lhsT=ones1[:, 0:rows],
                    rhs=k2neg_sb,
                    start=False,
                    stop=True,
                )
                e = work.tile([128, T2], f32, tag="e")
                s = small.tile([128, 1], f32, tag="s")
                nc.scalar.activation(
                    e[0:rows], a_ps[0:rows], AF.Exp, accum_out=s[0:rows]
                )
                r = small.tile([128, 1], f32, tag="r")
                nc.vector.reciprocal(r[0:rows], s[0:rows])
                ep = work.tile([128, T2], f32, tag="ep")
                nc.vector.tensor_mul(ep[0:rows], e[0:rows], prior_sb[0:rows, c, :])
                o1 = work.tile([128, T2], f32, tag="o1")
                nc.scalar.activation(o1[0:rows], ep[0:rows], AF.Ln, scale=r[0:rows])
                nc.sync.dma_start(out=ologp_p[r0 : r0 + rows, :], in_=o1[0:rows])
                wm = work.tile([128, T2], f32, tag="wm")
                nc.gpsimd.tensor_mul(wm[0:rows], ep[0:rows], pmask_bc[0:rows])
                s2 = small.tile([128, 1], f32, tag="s2")
                nc.vector.reduce_sum(s2[0:rows], wm[0:rows], axis=mybir.AxisListType.X)
                r2 = small.tile([128, 1], f32, tag="r2")
                nc.vector.reciprocal(r2[0:rows], s2[0:rows])
                o2 = work.tile([128, T2], f32, tag="o2")
                r2b = bass.AP(
                    tensor=r2.tensor,
                    offset=r2.offset,
                    ap=[list(r2.ap[0])[:1] + [rows], [0, T2]],
                )
                nc.vector.tensor_mul(o2[0:rows], wm[0:rows], r2b)
                nc.sync.dma_start(out=oattn_p[r0 : r0 + rows, :], in_=o2[0:rows])

    _split_excess_waits(nc)
    return nc


_NC_CACHE = {}


def _get_nc():
    if "nc" not in _NC_CACHE:
        _NC_CACHE["nc"] = _build_nc()
    return _NC_CACHE["nc"]


def _prep_in_maps(inputs):
    q = np.asarray(inputs["queries"], F32)
    k = np.asarray(inputs["keys"], F32)
    mask = np.asarray(inputs["mask"])
    prior = np.asarray(inputs["attn_prior"], F32)
    spk = np.asarray(inputs["speaker_embed"], F32)
    emo = np.asarray(inputs["emotion_embed"], F32)

    kw1 = np.asarray(inputs["kw1"], F32)
    kb1 = np.asarray(inputs["kb1"], F32)
    kw2 = np.asarray(inputs["kw2"], F32)
    kb2 = np.asarray(inputs["kb2"], F32)
    qw1 = np.asarray(inputs["qw1"], F32)
    qb1 = np.asarray(inputs["qb1"], F32)
    qw2 = np.asarray(inputs["qw2"], F32)
    qb2 = np.asarray(inputs["qb2"], F32)
    qw3 = np.asarray(inputs["qw3"], F32)
    qb3 = np.asarray(inputs["qb3"], F32)
    spk_kw = np.asarray(inputs["spk_kw"], F32)
    spk_kb = np.asarray(inputs["spk_kb"], F32)
    spk_qw = np.asarray(inputs["spk_qw"], F32)
    spk_qb = np.asarray(inputs["spk_qb"], F32)
    emo_kw = np.asarray(inputs["emo_kw"], F32)
    emo_kb = np.asarray(inputs["emo_kb"], F32)
    emo_qw = np.asarray(inputs["emo_qw"], F32)
    emo_qb = np.asarray(inputs["emo_qb"], F32)

    # concatenated speaker/emotion projection:  [k_add; q_add] = Wcat @ [spk; emo] + bvec
    wcat = np.concatenate(
        [
            np.concatenate([spk_kw, emo_kw], axis=1),
            np.concatenate([spk_qw, emo_qw], axis=1),
        ],
        axis=0,
    )  # (592, 1024)
    wcat_t = np.ascontiguousarray(
        wcat.T.reshape(8, 128, 592).transpose(1, 0, 2)
    ).astype(BF16)
    bcat = np.zeros(640, F32)
    bcat[0:TXT] = spk_kb + emo_kb
    bcat[TXT : TXT + MEL] = spk_qb + emo_qb
    bvec_t = np.ascontiguousarray(bcat.reshape(5, 128).T)
    z_all = np.concatenate([spk, emo], axis=1)  # (8, 1024)

    kw1_t = np.ascontiguousarray(
        kw1.reshape(8, 128, 4, 128, 3).transpose(3, 0, 2, 4, 1)
    ).astype(BF16)  # (j, co, ci, k, c)
    kw2_t = np.ascontiguousarray(
        kw2[:, :, 0].T.reshape(8, 128, ATT).transpose(1, 0, 2)
    ).astype(BF16)  # (j, ci2, c)
    kb1_t = np.ascontiguousarray(kb1.reshape(8, 128).T)
    qw1_t = np.ascontiguousarray(qw1.transpose(1, 2, 0)).astype(BF16)  # (j, k, c)
    qw2_t = np.ascontiguousarray(qw2[:, :, 0].T).astype(BF16)  # (160, 80)
    qw3_t = np.ascontiguousarray(qw3[:, :, 0].T).astype(BF16)  # (80, 80)

    shared = {
        "wcat": wcat_t,
        "bvec": bvec_t,
        "kw1": kw1_t,
        "kw2": kw2_t,
        "kb1": kb1_t,
        "kb2": np.ascontiguousarray(kb2.reshape(ATT, 1)),
        "qw1": qw1_t,
        "qw2": qw2_t,
        "qw3": qw3_t,
        "qb1": np.ascontiguousarray(qb1.reshape(160, 1)),
        "qb2": np.ascontiguousarray(qb2.reshape(MEL, 1)),
        "qb3": np.ascontiguousarray(qb3.reshape(MEL, 1)),
    }

    in_maps = []
    for b in range(B):
        m = dict(shared)
        m["queries"] = np.ascontiguousarray(q[b]).astype(BF16)
        m["keys"] = np.ascontiguousarray(k[b].reshape(4, 128, T2)).astype(BF16)
        m["prior"] = np.ascontiguousarray(prior[b])
        m["pmask"] = np.ascontiguousarray(
            1.0 - mask[b, :, 0].astype(F32)
        )
        m["z"] = np.ascontiguousarray(z_all[b].reshape(8, 128).T).astype(BF16)
        in_maps.append(m)
    return in_maps


def kernel(**inputs):
    from concourse.bass_utils import run_bass_kernel_spmd

    nc = _get_nc()
    in_maps = _prep_in_maps(inputs)
    res = run_bass_kernel_spmd(nc, in_maps, core_ids=list(range(N_CORES)))
    attn = np.stack([res.results[i]["out_attn"] for i in range(N_CORES)])
    logp = np.stack([res.results[i]["out_logp"] for i in range(N_CORES)])
    return attn[:, None].astype(F32), logp[:, None].astype(F32)
